# revision 44
# baseline (speedup 1.0000x reference)
"""Trainium2 Bass kernel for nn_KernelEncoderLayer (gnn_message_passing).

Math (per graph b of 4, N=1024 points, K=9 kernel offsets, C=32 channels):
  y[i,c] = leaky( sum_{n,k} exp(-|pi - pn - kk|^2/2) * (w @ conv_w[k])[n,c] )
  out = BN(y)+w -> MLP(32->128->32, leaky, BN) residual.

Factorization (k-independent Gaussian Gram matrix):
  exp(-|pi - pn - kk|^2/2) = G[n,i] * A[i,k] * B[n,k]
  G[n,i] = exp(pn.pi - |pn|^2/2 - |pi|^2/2)
  A[i,k] = exp(pi.kk),  B[n,k] = exp(-pn.kk - |kk|^2/2)   (host-precomputed)
So y[i,c] = sum_k A[i,k] * (G.T @ (B[:,k] * cw[:,k,:]))[i,c].

v2 performance structure:
  - G's pn.pi matmul runs in bf16 with a hi/lo split of the coordinates
    (contract dim 8) -> full tensor-engine rate with ~fp32 exponent accuracy.
  - All matmuls bf16 (1 cyc/row, fp32 PSUM); inputs packed into 5 DMAs.
  - B and A factors are host-replicated across the 32 channels so the
    (n,k)- and (i,k)-scalings are a few big vector ops, not 100+ tiny ones.
  - Two pipelined AllGathers ship each core's yT slice + BN0 partial sums
    (384-col half + stats first, 128-col half second) so the BN0 chain and
    most MLP1 work overlap the second collective; every core redundantly
    computes full-batch h (for BN1 stats) and its own output slice fully
    locally (no rank-dependent indexing).
  - MLP1's weight-side matmuls accumulate into held PSUM banks during the
    collectives; one PSUM accumulation group per bank (has_written clears
    are bank-granular).
  - Leaky uses Prelu and BN rsqrt uses Sqrt+recip: parametric_relu and
    sqrt live in the same ACT table set, so after the preload during the
    collective there are zero table switches post-gather.

Sharding: 8 cores = 4 graphs x 2 halves of the 1024 output rows. Each core
computes its [512, 32] conv-output slice and emits out[32, 512]; the host
concatenates.

Self-contained: hardcodes B=4, N=1024, K=9, C=32, CM=128, 8 cores.
"""

import numpy as np
import ml_dtypes

import concourse.bass as bass
import concourse.bacc as bacc
import concourse.mybir as mybir
import concourse.tile as tile
from concourse import masks
from concourse.bass_utils import run_bass_kernel_spmd

F32 = mybir.dt.float32
F32R = mybir.dt.float32r
BF16 = mybir.dt.bfloat16
I32 = mybir.dt.int32
AF = mybir.ActivationFunctionType
ALU = mybir.AluOpType
AX = mybir.AxisListType

NB, N, K, C, CM = 4, 1024, 9, 32, 128
NCORES = 8
EPS = 1e-5
SLOPE = 0.01
NT = NB * N
KC = K * C  # 288
MAGIC = 0x5F3759DF

WARM_CC = False  # dummy CC measured as pure serial overhead here


def _r(ap):
    return ap.bitcast(F32R)


def _build_module():
    nc = bacc.Bacc("TRN2", target_bir_lowering=False, debug=False,
                   num_devices=NCORES)

    def din(name, shape, dt=F32):
        return nc.dram_tensor(name, list(shape), dt, kind="ExternalInput").ap()

    # packed inputs (few big DMAs): see _host_prep for layouts
    packp_d = din("packp", (8, 1536), BF16)   # pn8 | pi8 hi/lo coord rows
    packw_d = din("packw", (C, 6048), BF16)   # wTb|wtext|wtob|w1b16|cwall
    packf_d = din("packf", (128, 3456), BF16)  # bexp | aexp (c-replicated)
    packs_d = din("packs", (C + 1, 644))      # w1e | g0,be0,b2 | wto
    p128_d = din("p128", (128, 44))           # negsqn(8)|w2(32)|g1|be1|pad

    out_d = nc.dram_tensor("out", [C, 512], F32, kind="ExternalOutput").ap()

    with tile.TileContext(nc) as tc:
        with (
            tc.tile_pool(name="const", bufs=1) as pc,
            tc.tile_pool(name="big", bufs=1) as pb,
            tc.tile_pool(name="work", bufs=3) as pw,
            tc.tile_pool(name="dram", bufs=1, space="DRAM") as pd,
        ):
            # ---- optional collective warmup (content irrelevant) ----
            if WARM_CC:
                dmy_in = pd.tile([1, 8], F32, tag="dmy_in")
                dmy_out = pd.tile([NCORES, 8], F32, tag="dmy_out")
                nc.gpsimd.collective_compute(
                    "AllGather", ALU.bypass,
                    replica_groups=[list(range(NCORES))],
                    ins=[dmy_in[:].opt()], outs=[dmy_out[:].opt()])

            # ---- input loads ----
            def load(name, ap, shape, dt=F32, pool=pc):
                t = pool.tile(list(shape), dt, tag=name, name=name)
                nc.sync.dma_start(out=t[:], in_=ap)
                return t

            p128 = load("p128", p128_d, (128, 44))
            packp = load("packp", packp_d, (8, 1536), BF16)
            packw = load("packw", packw_d, (C, 6048), BF16, pool=pb)
            packf = load("packf", packf_d, (128, 3456), BF16, pool=pb)
            packs = load("packs", packs_d, (C + 1, 644))

            pn8 = packp[:, 0:N]
            pi8 = packp[:, N:N + 512]
            wTb = packw[:, 0:1024]
            wtext = packw[:, 1024:5120]
            wtob = packw[:, 5120:5632]
            w1b16 = packw[:, 5632:5760]
            cwall = packw[:, 5760:6048]
            bexp = packf[:, 0:8 * KC]
            aexp = packf[:, 8 * KC:12 * KC]
            w1e = packs[:, 0:CM]
            g0c = packs[0:C, 128:129]
            be0c = packs[0:C, 129:130]
            b2c = packs[0:C, 130:131]
            wto = packs[0:C, 131:643]

            negsqn = p128[:, 0:8]
            w2 = p128[:, 8:40]
            g1c = p128[:, 40:41]
            be1c = p128[:, 41:42]

            ident = pc.tile([128, 128], F32, tag="ident")
            masks.make_identity(nc, ident[:])
            warm0 = pc.tile([128, 1], F32, tag="warm0")
            nc.scalar.activation(warm0[:], ident[:, 0:1], AF.Exp)

            ag_y = pb.tile([C, 516], BF16, tag="ag_y")
            ysum_p = pc.tile([C, 4], F32, tag="ysum")
            ysq_p = pc.tile([C, 4], F32, tag="ysq")

            with (
                tc.tile_pool(name="psG", bufs=2, space="PSUM") as psG,
                tc.tile_pool(name="psB", bufs=2, space="PSUM") as psB,
                tc.tile_pool(name="psD", bufs=2, space="PSUM") as psD,
                tc.tile_pool(name="psC", bufs=1, space="PSUM") as psC,
            ):
                # ---- G[n,i] = exp(pn.pi - |pn|^2/2 - |pi|^2/2), bf16 hi/lo
                g_sb = [pb.tile([128, 512], BF16, tag=f"g{j}", name=f"g{j}")
                        for j in range(8)]
                # ---- cw'[n,(k,c)] = B[n,k] * (w @ conv_w[k]), bf16
                cw_sb = [pb.tile([128, KC], BF16, tag=f"cw{j}", name=f"cw{j}")
                         for j in range(8)]
                for j in range(8):
                    psg = psG.tile([128, 512], F32, tag="g")
                    nc.tensor.matmul(psg[:], lhsT=pn8[:, j * 128:(j + 1) * 128],
                                     rhs=pi8, start=True, stop=True)
                    nc.scalar.activation(g_sb[j][:], psg[:], AF.Exp,
                                         bias=negsqn[:, j:j + 1], scale=1.0)
                    psb = psB.tile([128, KC], F32, tag="b")
                    nc.tensor.matmul(psb[:], lhsT=wTb[:, j * 128:(j + 1) * 128],
                                     rhs=cwall, start=True, stop=True)
                    nc.vector.tensor_tensor(
                        cw_sb[j][:], psb[:],
                        bexp[:, j * KC:(j + 1) * KC], op=ALU.mult)

                # ---- main contraction + combine, two t's at a time ----
                # (2 PSUM accumulator banks; pair 0-1 combines while pair
                # 2-3 accumulates)
                for tp in range(2):
                    pys = [psD.tile([128, KC], F32, tag="py",
                                    name=f"py{tp}_{ti}")
                           for ti in range(2)]
                    for j in range(8):
                        for ti in range(2):
                            t = tp * 2 + ti
                            nc.tensor.matmul(
                                pys[ti][:],
                                lhsT=g_sb[j][:, t * 128:(t + 1) * 128],
                                rhs=cw_sb[j][:],
                                start=(j == 0), stop=(j == 7))
                    for ti in range(2):
                        t = tp * 2 + ti
                        ya = pw.tile([128, KC], F32, tag="ya")
                        nc.vector.tensor_tensor(
                            ya[:], pys[ti][:], aexp[:, t * KC:(t + 1) * KC],
                            op=ALU.mult)
                        y_t = pw.tile([128, C], F32, tag="yt")
                        nc.vector.tensor_reduce(
                            y_t[:], ya[:].rearrange("p (k c) -> p c k", k=K),
                            axis=AX.X, op=ALU.add)
                        y_l = pw.tile([128, C], F32, tag="yl")
                        nc.vector.scalar_tensor_tensor(
                            y_l[:], y_t[:], SLOPE, y_t[:],
                            op0=ALU.mult, op1=ALU.max)
                        ptr = psC.tile([C, 128], F32, tag="tr")
                        nc.tensor.transpose(ptr[:], y_l[:], ident[:])
                        tc0 = t * 128 if t < 3 else 388
                        nc.vector.tensor_scalar(
                            ag_y[:, tc0:tc0 + 128], ptr[:], 0.0, 0.0,
                            op0=ALU.add, op1=ALU.add,
                            accum_out=ysum_p[:, t:t + 1])
                        agt = ag_y[:, tc0:tc0 + 128]
                        sq = pw.tile([C, 128], BF16, tag="sq")
                        nc.vector.scalar_tensor_tensor(
                            sq[:], agt, 1.0, agt, op0=ALU.mult, op1=ALU.mult,
                            accum_out=ysq_p[:, t:t + 1])
                stat_cols = ag_y[:, 384:388].bitcast(F32)
                nc.vector.tensor_reduce(stat_cols[:, 0:1], ysum_p[:],
                                        axis=AX.X, op=ALU.add)
                nc.vector.tensor_reduce(stat_cols[:, 1:2], ysq_p[:],
                                        axis=AX.X, op=ALU.add)

            with (
                tc.tile_pool(name="psH", bufs=1, space="PSUM") as psH,
                tc.tile_pool(name="psR", bufs=2, space="PSUM") as psR,
                tc.tile_pool(name="psT", bufs=1, space="PSUM") as psT,
            ):
                # ---- during the collective: w-side of MLP1 for all chunks
                NHOLD = 5
                hold = [psH.tile([128, 512], F32, tag=f"h{ci}", name=f"h{ci}")
                        for ci in range(NHOLD)]
                hold += [psR.tile([128, 512], F32, tag="hr", name=f"h{ci}")
                         for ci in range(NHOLD, 7)]
                HB = [0, 384, 512]
                with tc.high_priority():
                    for ci in range(7):
                        for hf in range(2):
                            nc.tensor.matmul(
                                hold[ci][:, HB[hf]:HB[hf + 1]],
                                lhsT=w1b16,
                                rhs=wtext[:, ci * 512 + HB[hf]:
                                          ci * 512 + HB[hf + 1]],
                                start=(hf == 0), stop=False)
                    # preload the Sqrt table while the collective runs
                    warm = pw.tile([128, 1], F32, tag="warm")
                    nc.scalar.activation(warm[:], p128[:, 42:43], AF.Sqrt)

                # ---- two pipelined AllGathers: [y-left+stats], [y-right]
                agA_in = pd.tile([C, 388], BF16, tag="agA_in")
                agA_out = pd.tile([C * NCORES, 388], BF16, tag="agA_out")
                agB_in = pd.tile([C, 132], BF16, tag="agB_in")
                agB_out = pd.tile([C * NCORES, 132], BF16, tag="agB_out")
                nc.sync.dma_start(out=agA_in[:], in_=ag_y[:, 0:388])
                # B also reads the stats cols (384:388) so both collectives
                # become ready together and emission order decides: A first.
                nc.sync.dma_start(out=agB_in[:], in_=ag_y[:, 384:516])
                nc.gpsimd.collective_compute(
                    "AllGather", ALU.bypass,
                    replica_groups=[list(range(NCORES))],
                    ins=[agA_in[:].opt()], outs=[agA_out[:].opt()])
                nc.gpsimd.collective_compute(
                    "AllGather", ALU.bypass,
                    replica_groups=[list(range(NCORES))],
                    ins=[agB_in[:].opt()], outs=[agB_out[:].opt()])

                # ---- receive the gathers ----
                agvA = agA_out[:].rearrange("(jj p) n -> p jj n", jj=NCORES)
                agvB = agB_out[:].rearrange("(jj p) n -> p jj n", jj=NCORES)
                statj = pc.tile([C, 32], BF16, tag="statj")
                nc.sync.dma_start(
                    out=statj[:].rearrange("p (jj s) -> p jj s", s=4),
                    in_=agvA[:, :, 384:388])
                yt_full = pb.tile([C, NT], BF16, tag="yt_full")
                ytv = yt_full[:].rearrange("p (ci n) -> p ci n", ci=NCORES)
                nc.sync.dma_start(out=ytv[:, :, 0:384],
                                  in_=agvA[:, :, 0:384])
                nc.sync.dma_start(out=ytv[:, :, 384:512],
                                  in_=agvB[:, :, 4:132])

                # ---- BN0 global stats ----
                statv = statj[:].bitcast(F32).rearrange("p (jj s) -> p s jj", s=2)
                tot = pc.tile([C, 2], F32, tag="tot")
                nc.vector.tensor_reduce(tot[:, 0:1], statv[:, 0:1, :],
                                        axis=AX.X, op=ALU.add)
                nc.vector.tensor_reduce(tot[:, 1:2], statv[:, 1:2, :],
                                        axis=AX.X, op=ALU.add)
                mom = pc.tile([C, 2], F32, tag="mom")
                nc.vector.tensor_scalar_mul(mom[:], tot[:], 1.0 / NT)
                var0 = pc.tile([C, 1], F32, tag="var0")
                nc.vector.tensor_tensor(var0[:], mom[:, 0:1], mom[:, 0:1],
                                        op=ALU.mult)
                nc.vector.scalar_tensor_tensor(
                    var0[:], mom[:, 1:2], EPS, var0[:],
                    op0=ALU.add, op1=ALU.subtract)

                def rsqrt(dst, src, p, pref):
                    # ACT Sqrt (table preloaded during the AG) + DVE recip
                    st = pc.tile([p, 1], F32, tag=pref + "s")
                    nc.scalar.activation(st[:], src[:], AF.Sqrt)
                    nc.vector.reciprocal(dst[:], st[:])

                rstd0 = pc.tile([C, 1], F32, tag="rstd0")
                rsqrt(rstd0, var0, C, "r0")
                scale0 = pc.tile([C, 1], F32, tag="scale0")
                nc.vector.tensor_tensor(scale0[:], rstd0[:], g0c,
                                        op=ALU.mult)
                w1p = pc.tile([C, CM], BF16, tag="w1p")
                nc.vector.tensor_scalar(w1p[:], w1e[0:C, :], scale0[:], None,
                                        op0=ALU.mult)
                nshift0 = pc.tile([C, 1], F32, tag="nshift0")
                nc.vector.scalar_tensor_tensor(
                    nshift0[:], mom[:, 0:1], scale0[:], be0c,
                    op0=ALU.mult, op1=ALU.subtract)

                # bias1 = w1.T @ shift0 + b1  (ones-row trick via w1e row C)
                sh1e = pc.tile([C + 1, 1], F32, tag="sh1e")
                nc.vector.memset(sh1e[C:C + 1, :], -1.0)
                nc.vector.tensor_copy(out=sh1e[0:C, :], in_=nshift0[:])
                psb1 = psT.tile([CM, 1], F32, tag="misc", name="psb1")
                nc.tensor.matmul(psb1[:], lhsT=w1e, rhs=sh1e[:],
                                 start=True, stop=True)
                bias1 = pc.tile([CM, 1], F32, tag="bias1")
                nc.vector.tensor_scalar_mul(bias1[:], psb1[:], -1.0)

                # ---- full-batch h chunks: leaky(w1'.T yT + w1.T wT + bias1)
                # processed in halves so left halves overlap AG-B
                hstat = pc.tile([CM, 32], F32, tag="hstat")
                for hf in range(2):
                    for ci in range(8):
                        if ci < 7:
                            ps = hold[ci]
                        else:
                            if hf == 0:
                                ps = psT.tile([128, 512], F32, tag="misc",
                                              name="h7")
                                hold.append(ps)
                            else:
                                ps = hold[7]
                            nc.tensor.matmul(
                                ps[:, HB[hf]:HB[hf + 1]], lhsT=w1b16,
                                rhs=wtext[:, ci * 512 + HB[hf]:
                                          ci * 512 + HB[hf + 1]],
                                start=(hf == 0), stop=False)
                        cl, cr = ci * 512 + HB[hf], ci * 512 + HB[hf + 1]
                        nc.tensor.matmul(
                            ps[:, HB[hf]:HB[hf + 1]], lhsT=w1p[:],
                            rhs=yt_full[:, cl:cr],
                            start=False, stop=(hf == 1))
                        sc = ci * 2 + hf
                        hcs = pw.tile([CM, 384], BF16, tag="hcs",
                                      name=f"hcs{sc}")
                        hcs_a = hcs[:, 0:HB[hf + 1] - HB[hf]]
                        if hf == 1 and ci % 2 == 1:
                            # spread the AG-B-gated leakys across ACT+DVE:
                            # leaky(x+b) = max(x+b, slope*(x+b)) via two-op
                            pre = pw.tile([CM, 128], F32, tag="pre",
                                          name=f"pre{sc}")
                            nc.vector.tensor_scalar(
                                pre[:], ps[:, HB[hf]:HB[hf + 1]], bias1[:],
                                None, op0=ALU.add)
                            nc.vector.scalar_tensor_tensor(
                                hcs_a, pre[:], SLOPE, pre[:],
                                op0=ALU.mult, op1=ALU.max,
                                accum_out=hstat[:, sc:sc + 1])
                        else:
                            nc.scalar.activation(
                                hcs_a, ps[:, HB[hf]:HB[hf + 1]],
                                AF.Prelu, bias=bias1[:],
                                scale=1.0, alpha=SLOPE,
                                accum_out=hstat[:, sc:sc + 1])
                        sqh = pw.tile([CM, 384], BF16, tag="sqh",
                                      name=f"sqh{sc}")
                        sqh_a = sqh[:, 0:HB[hf + 1] - HB[hf]]
                        nc.vector.scalar_tensor_tensor(
                            sqh_a, hcs_a, 1.0, hcs_a,
                            op0=ALU.mult, op1=ALU.mult,
                            accum_out=hstat[:, 16 + sc:17 + sc])

                # ---- own-slice yres + h (fully local, no rank indexing) ----
                yres = pb.tile([C, 512], F32, tag="yres")
                with tc.high_priority():
                    nc.vector.tensor_scalar(yres[:, 0:384], ag_y[:, 0:384],
                                            scale0[:], nshift0[:],
                                            op0=ALU.mult, op1=ALU.subtract)
                    nc.vector.tensor_scalar(yres[:, 384:512],
                                            ag_y[:, 388:516],
                                            scale0[:], nshift0[:],
                                            op0=ALU.mult, op1=ALU.subtract)
                    nc.vector.tensor_tensor(yres[:], yres[:], wto,
                                            op=ALU.add)
                pso = psR.tile([128, 512], F32, tag="hr")
                nc.tensor.matmul(pso[:], lhsT=w1b16, rhs=wtob,
                                 start=True, stop=False)
                nc.tensor.matmul(pso[:, 0:384], lhsT=w1p[:],
                                 rhs=ag_y[:, 0:384],
                                 start=False, stop=False)
                nc.tensor.matmul(pso[:, 384:512], lhsT=w1p[:],
                                 rhs=ag_y[:, 388:516],
                                 start=False, stop=True)
                h_own = pb.tile([CM, 512], BF16, tag="h_own")
                nc.scalar.activation(h_own[:], pso[:], AF.Prelu,
                                     bias=bias1[:], scale=1.0, alpha=SLOPE)

                # ---- BN1 stats + fold into w2 ----
                tot1 = pc.tile([CM, 2], F32, tag="tot1")
                nc.vector.tensor_reduce(tot1[:, 0:1], hstat[:, 0:16],
                                        axis=AX.X, op=ALU.add)
                nc.vector.tensor_reduce(tot1[:, 1:2], hstat[:, 16:32],
                                        axis=AX.X, op=ALU.add)
                mom1 = pc.tile([CM, 2], F32, tag="mom1")
                nc.vector.tensor_scalar_mul(mom1[:], tot1[:], 1.0 / NT)
                var1 = pc.tile([CM, 1], F32, tag="var1")
                nc.vector.tensor_tensor(var1[:], mom1[:, 0:1], mom1[:, 0:1],
                                        op=ALU.mult)
                nc.vector.scalar_tensor_tensor(
                    var1[:], mom1[:, 1:2], EPS, var1[:],
                    op0=ALU.add, op1=ALU.subtract)
                rstd1 = pc.tile([CM, 1], F32, tag="rstd1")
                rsqrt(rstd1, var1, CM, "r1")
                scale1 = pc.tile([CM, 1], F32, tag="scale1")
                nc.vector.tensor_tensor(scale1[:], rstd1[:], g1c,
                                        op=ALU.mult)
                nshift1 = pc.tile([CM, 1], F32, tag="nshift1")
                nc.vector.scalar_tensor_tensor(
                    nshift1[:], mom1[:, 0:1], scale1[:], be1c,
                    op0=ALU.mult, op1=ALU.subtract)
                w2p = pc.tile([CM, C], BF16, tag="w2p")
                nc.vector.tensor_scalar(w2p[:], w2, scale1[:], None,
                                        op0=ALU.mult)
                psb2 = psT.tile([C, 1], F32, tag="misc", name="psb2")
                nc.tensor.matmul(psb2[:], lhsT=w2, rhs=nshift1[:],
                                 start=True, stop=True)
                bias2 = pc.tile([C, 1], F32, tag="bias2")
                nc.vector.tensor_tensor(bias2[:], b2c, psb2[:],
                                        op=ALU.subtract)

                # ---- delta = w2'.T @ h_own + bias2; out = yres + delta ----
                psd = psT.tile([C, 512], F32, tag="misc", name="psd")
                nc.tensor.matmul(psd[:], lhsT=w2p[:], rhs=h_own[:],
                                 start=True, stop=True)
                out_sb = pw.tile([C, 512], F32, tag="outsb")
                nc.vector.scalar_tensor_tensor(
                    out_sb[:], psd[:], bias2[:], yres[:],
                    op0=ALU.add, op1=ALU.add)
                nc.scalar.dma_start(out=out_d, in_=out_sb[:])


    nc.compile()
    return nc


_NC_CACHE = {}


def _get_module():
    if "nc" not in _NC_CACHE:
        _NC_CACHE["nc"] = _build_module()
    return _NC_CACHE["nc"]


BF = ml_dtypes.bfloat16


def _split_hi_lo(x):
    hi = x.astype(BF)
    lo = (x - hi.astype(np.float32)).astype(BF)
    return hi, lo


def _host_prep(inputs):
    pos = np.asarray(inputs["positions"], np.float32)
    w = np.asarray(inputs["weights"], np.float32)
    kp = np.asarray(inputs["kernel_pos"], np.float32)
    cw = np.asarray(inputs["conv_w"], np.float32)
    posb = pos.reshape(NB, N, 2)
    wb = w.reshape(NB, N, C)
    kk2 = 0.5 * (kp ** 2).sum(1)                       # [9]
    cwall = cw.transpose(1, 0, 2).reshape(C, KC).astype(BF)
    wTfull = np.ascontiguousarray(w.T)                 # [32, 4096]
    w1 = np.asarray(inputs["w1"], np.float32)
    w2 = np.asarray(inputs["w2"], np.float32)
    w1e = np.concatenate(
        [w1, np.asarray(inputs["b1"], np.float32).reshape(1, CM)], axis=0)
    p128 = np.zeros((128, 44), np.float32)
    p128[:, 8:40] = w2
    p128[:, 40] = np.asarray(inputs["bn1_gamma"], np.float32)
    p128[:, 41] = np.asarray(inputs["bn1_beta"], np.float32)
    p128[:, 42] = np.int32(1).view(np.float32)
    p128[:, 43] = np.int32(MAGIC).view(np.float32)

    in_maps = []
    for j in range(NCORES):
        b, off = j // 2, (j % 2) * 512
        p = posb[b]
        pi = p[off:off + 512]
        xh, xl = _split_hi_lo(p[:, 0])
        yh, yl = _split_hi_lo(p[:, 1])
        one = np.ones(N, BF)
        pn8 = np.stack([xh, xh, xl, yh, yh, yl, one, one])
        bias = -0.5 * (pi ** 2).sum(1)
        bh, bl = _split_hi_lo(bias)
        xih, xil = _split_hi_lo(pi[:, 0])
        yih, yil = _split_hi_lo(pi[:, 1])
        pi8 = np.stack([xih, xil, xih, yih, yil, yih, bh, bl])
        packp = np.concatenate([pn8, pi8], axis=1)          # [8, 1536]

        packw = np.empty((C, 6048), BF)
        packw[:, 0:1024] = wb[b].T.astype(BF)
        packw[:, 1024:5120] = wTfull.astype(BF)
        packw[:, 5120:5632] = wTfull[:, j * 512:(j + 1) * 512].astype(BF)
        packw[:, 5632:5760] = w1.astype(BF)
        packw[:, 5760:6048] = cwall

        p128j = p128.copy()
        p128j[:, 0:8] = (-0.5 * (p ** 2).sum(1)).reshape(8, 128).T
        dotn = (p @ kp.T).astype(np.float32)            # [1024, 9]
        Bmat = np.exp(-dotn - kk2[None, :]).astype(np.float32)
        Amat = np.exp((pi @ kp.T).astype(np.float32)).astype(np.float32)
        packf = np.empty((128, 3456), BF)
        packf[:, 0:8 * KC] = np.broadcast_to(
            Bmat.reshape(8, 128, K, 1).transpose(1, 0, 2, 3),
            (128, 8, K, C)).reshape(128, 8 * KC).astype(BF)
        packf[:, 8 * KC:] = np.broadcast_to(
            Amat.reshape(4, 128, K, 1).transpose(1, 0, 2, 3),
            (128, 4, K, C)).reshape(128, 4 * KC).astype(BF)

        packs = np.zeros((C + 1, 644), np.float32)
        packs[:, 0:CM] = w1e
        packs[0:C, 128] = np.asarray(inputs["bn_gamma"], np.float32)
        packs[0:C, 129] = np.asarray(inputs["bn_beta"], np.float32)
        packs[0:C, 130] = np.asarray(inputs["b2"], np.float32)
        packs[0:C, 131:643] = wTfull[:, j * 512:(j + 1) * 512]

        in_maps.append(dict(
            packp=np.ascontiguousarray(packp),
            packw=np.ascontiguousarray(packw),
            packf=np.ascontiguousarray(packf),
            packs=np.ascontiguousarray(packs),
            p128=np.ascontiguousarray(p128j)))
    return in_maps


def _run(inputs, trace=False):
    nc = _get_module()
    in_maps = _host_prep(inputs)
    res = run_bass_kernel_spmd(nc, in_maps, core_ids=list(range(NCORES)),
                               trace=trace)
    out = np.concatenate([np.asarray(res.results[j]["out"])
                          for j in range(NCORES)], axis=1)   # [32, 4096]
    return np.ascontiguousarray(out.T), res


def kernel(**inputs):
    out, _ = _run(inputs, trace=False)
    return out


# revision 45
# speedup vs baseline: 1.1686x; 1.1686x over previous
"""Trainium2 Bass kernel for nn_KernelEncoderLayer (gnn_message_passing).

Math (per graph b of 4, N=1024 points, K=9 kernel offsets, C=32 channels):
  y[i,c] = leaky( sum_{n,k} exp(-|pi - pn - kk|^2/2) * (w @ conv_w[k])[n,c] )
  out = BN(y)+w -> MLP(32->128->32, leaky, BN) residual.

Factorization (k-independent Gaussian Gram matrix):
  exp(-|pi - pn - kk|^2/2) = G[n,i] * A[i,k] * B[n,k]
  G[n,i] = exp(pn.pi - |pn|^2/2 - |pi|^2/2)
  A[i,k] = exp(pi.kk),  B[n,k] = exp(-pn.kk - |kk|^2/2)   (host-precomputed)
So y[i,c] = sum_k A[i,k] * (G.T @ (B[:,k] * cw[:,k,:]))[i,c].

v2 performance structure:
  - G's pn.pi matmul runs in bf16 with a hi/lo split of the coordinates
    (contract dim 8) -> full tensor-engine rate with ~fp32 exponent accuracy.
  - All matmuls bf16 (1 cyc/row, fp32 PSUM); inputs packed into 5 DMAs.
  - B and A factors are host-replicated across the 32 channels so the
    (n,k)- and (i,k)-scalings are a few big vector ops, not 100+ tiny ones.
  - Two pipelined AllGathers ship each core's yT slice + BN0 partial sums
    (384-col half + stats first, 128-col half second) so the BN0 chain and
    most MLP1 work overlap the second collective; every core redundantly
    computes full-batch h (for BN1 stats) and its own output slice fully
    locally (no rank-dependent indexing).
  - MLP1's weight-side matmuls accumulate into held PSUM banks during the
    collectives; one PSUM accumulation group per bank (has_written clears
    are bank-granular).
  - Leaky uses Prelu and BN rsqrt uses Sqrt+recip: parametric_relu and
    sqrt live in the same ACT table set, so after the preload during the
    collective there are zero table switches post-gather.

Sharding: 8 cores = 4 graphs x 2 halves of the 1024 output rows. Each core
computes its [512, 32] conv-output slice and emits out[32, 512]; the host
concatenates.

Self-contained: hardcodes B=4, N=1024, K=9, C=32, CM=128, 8 cores.
"""

import numpy as np
import ml_dtypes

import concourse.bass as bass
import concourse.bacc as bacc
import concourse.mybir as mybir
import concourse.tile as tile
from concourse import masks
from concourse.bass_utils import run_bass_kernel_spmd

F32 = mybir.dt.float32
F32R = mybir.dt.float32r
BF16 = mybir.dt.bfloat16
I32 = mybir.dt.int32
AF = mybir.ActivationFunctionType
ALU = mybir.AluOpType
AX = mybir.AxisListType

NB, N, K, C, CM = 4, 1024, 9, 32, 128
NCORES = 8
EPS = 1e-5
SLOPE = 0.01
NT = NB * N
KC = K * C  # 288
MAGIC = 0x5F3759DF

WARM_CC = False  # dummy CC measured as pure serial overhead here


def _r(ap):
    return ap.bitcast(F32R)


def _build_module():
    nc = bacc.Bacc("TRN2", target_bir_lowering=False, debug=False,
                   num_devices=NCORES)

    def din(name, shape, dt=F32):
        return nc.dram_tensor(name, list(shape), dt, kind="ExternalInput").ap()

    # packed inputs (few big DMAs): see _host_prep for layouts
    packp_d = din("packp", (8, 1536), BF16)   # pn8 | pi8 hi/lo coord rows
    packw_d = din("packw", (C, 6048), BF16)   # wTb|wtext|wtob|w1b16|cwall
    packf_d = din("packf", (128, 3456), BF16)  # bexp | aexp (c-replicated)
    packs_d = din("packs", (C + 1, 644))      # w1e | g0,be0,b2 | wto
    p128_d = din("p128", (128, 44))           # negsqn(8)|w2(32)|g1|be1|pad

    out_d = nc.dram_tensor("out", [C, 512], F32, kind="ExternalOutput").ap()

    with tile.TileContext(nc) as tc:
        with (
            tc.tile_pool(name="const", bufs=1) as pc,
            tc.tile_pool(name="big", bufs=1) as pb,
            tc.tile_pool(name="work", bufs=3) as pw,
            tc.tile_pool(name="dram", bufs=1, space="DRAM") as pd,
        ):
            # ---- optional collective warmup (content irrelevant) ----
            if WARM_CC:
                dmy_in = pd.tile([1, 8], F32, tag="dmy_in")
                dmy_out = pd.tile([NCORES, 8], F32, tag="dmy_out")
                nc.gpsimd.collective_compute(
                    "AllGather", ALU.bypass,
                    replica_groups=[list(range(NCORES))],
                    ins=[dmy_in[:].opt()], outs=[dmy_out[:].opt()])

            # ---- input loads ----
            def load(name, ap, shape, dt=F32, pool=pc):
                t = pool.tile(list(shape), dt, tag=name, name=name)
                nc.sync.dma_start(out=t[:], in_=ap)
                return t

            p128 = load("p128", p128_d, (128, 44))
            packp = load("packp", packp_d, (8, 1536), BF16)
            packw = load("packw", packw_d, (C, 6048), BF16, pool=pb)
            packf = load("packf", packf_d, (128, 3456), BF16, pool=pb)
            packs = load("packs", packs_d, (C + 1, 644))

            pn8 = packp[:, 0:N]
            pi8 = packp[:, N:N + 512]
            wTb = packw[:, 0:1024]
            wtext = packw[:, 1024:5120]
            wtob = packw[:, 5120:5632]
            w1b16 = packw[:, 5632:5760]
            cwall = packw[:, 5760:6048]
            bexp = packf[:, 0:8 * KC]
            aexp = packf[:, 8 * KC:12 * KC]
            w1e = packs[:, 0:CM]
            g0c = packs[0:C, 128:129]
            be0c = packs[0:C, 129:130]
            b2c = packs[0:C, 130:131]
            wto = packs[0:C, 131:643]

            negsqn = p128[:, 0:8]
            w2 = p128[:, 8:40]
            g1c = p128[:, 40:41]
            be1c = p128[:, 41:42]

            ident = pc.tile([128, 128], F32, tag="ident")
            masks.make_identity(nc, ident[:])
            warm0 = pc.tile([128, 1], F32, tag="warm0")
            nc.scalar.activation(warm0[:], ident[:, 0:1], AF.Exp)

            ag_y = pb.tile([C, 516], BF16, tag="ag_y")
            ysum_p = pc.tile([C, 4], F32, tag="ysum")
            ysq_p = pc.tile([C, 4], F32, tag="ysq")

            with (
                tc.tile_pool(name="psG", bufs=2, space="PSUM") as psG,
                tc.tile_pool(name="psB", bufs=2, space="PSUM") as psB,
                tc.tile_pool(name="psD", bufs=2, space="PSUM") as psD,
                tc.tile_pool(name="psC", bufs=1, space="PSUM") as psC,
            ):
                # ---- G[n,i] = exp(pn.pi - |pn|^2/2 - |pi|^2/2), bf16 hi/lo
                g_sb = [pb.tile([128, 512], BF16, tag=f"g{j}", name=f"g{j}")
                        for j in range(8)]
                # ---- cw'[n,(k,c)] = B[n,k] * (w @ conv_w[k]), bf16
                cw_sb = [pb.tile([128, KC], BF16, tag=f"cw{j}", name=f"cw{j}")
                         for j in range(8)]
                for j in range(8):
                    psg = psG.tile([128, 512], F32, tag="g")
                    nc.tensor.matmul(psg[:], lhsT=pn8[:, j * 128:(j + 1) * 128],
                                     rhs=pi8, start=True, stop=True)
                    nc.scalar.activation(g_sb[j][:], psg[:], AF.Exp,
                                         bias=negsqn[:, j:j + 1], scale=1.0)
                    psb = psB.tile([128, KC], F32, tag="b")
                    nc.tensor.matmul(psb[:], lhsT=wTb[:, j * 128:(j + 1) * 128],
                                     rhs=cwall, start=True, stop=True)
                    nc.vector.tensor_tensor(
                        cw_sb[j][:], psb[:],
                        bexp[:, j * KC:(j + 1) * KC], op=ALU.mult)

                # ---- main contraction + combine, two t's at a time ----
                # (2 PSUM accumulator banks; pair 0-1 combines while pair
                # 2-3 accumulates)
                for tp in range(2):
                    pys = [psD.tile([128, KC], F32, tag="py",
                                    name=f"py{tp}_{ti}")
                           for ti in range(2)]
                    for j in range(8):
                        for ti in range(2):
                            t = tp * 2 + ti
                            nc.tensor.matmul(
                                pys[ti][:],
                                lhsT=g_sb[j][:, t * 128:(t + 1) * 128],
                                rhs=cw_sb[j][:],
                                start=(j == 0), stop=(j == 7))
                    for ti in range(2):
                        t = tp * 2 + ti
                        ya = pw.tile([128, KC], F32, tag="ya")
                        nc.vector.tensor_tensor(
                            ya[:], pys[ti][:], aexp[:, t * KC:(t + 1) * KC],
                            op=ALU.mult)
                        y_t = pw.tile([128, C], F32, tag="yt")
                        nc.vector.tensor_reduce(
                            y_t[:], ya[:].rearrange("p (k c) -> p c k", k=K),
                            axis=AX.X, op=ALU.add)
                        y_l = pw.tile([128, C], F32, tag="yl")
                        nc.vector.scalar_tensor_tensor(
                            y_l[:], y_t[:], SLOPE, y_t[:],
                            op0=ALU.mult, op1=ALU.max)
                        ptr = psC.tile([C, 128], F32, tag="tr")
                        nc.tensor.transpose(ptr[:], y_l[:], ident[:])
                        tc0 = t * 128 if t < 3 else 388
                        nc.vector.tensor_scalar(
                            ag_y[:, tc0:tc0 + 128], ptr[:], 0.0, 0.0,
                            op0=ALU.add, op1=ALU.add,
                            accum_out=ysum_p[:, t:t + 1])
                        agt = ag_y[:, tc0:tc0 + 128]
                        sq = pw.tile([C, 128], BF16, tag="sq")
                        nc.vector.scalar_tensor_tensor(
                            sq[:], agt, 1.0, agt, op0=ALU.mult, op1=ALU.mult,
                            accum_out=ysq_p[:, t:t + 1])
                stat_cols = ag_y[:, 384:388].bitcast(F32)
                nc.vector.tensor_reduce(stat_cols[:, 0:1], ysum_p[:],
                                        axis=AX.X, op=ALU.add)
                nc.vector.tensor_reduce(stat_cols[:, 1:2], ysq_p[:],
                                        axis=AX.X, op=ALU.add)

            with (
                tc.tile_pool(name="psH", bufs=1, space="PSUM") as psH,
                tc.tile_pool(name="psR", bufs=2, space="PSUM") as psR,
                tc.tile_pool(name="psT", bufs=1, space="PSUM") as psT,
            ):
                # ---- during the collective: w-side of MLP1 for all chunks
                NHOLD = 5
                hold = [psH.tile([128, 512], F32, tag=f"h{ci}", name=f"h{ci}")
                        for ci in range(NHOLD)]
                hold += [psR.tile([128, 512], F32, tag="hr", name=f"h{ci}")
                         for ci in range(NHOLD, 7)]
                HB = [0, 384, 512]
                with tc.high_priority():
                    for ci in range(7):
                        for hf in range(2):
                            nc.tensor.matmul(
                                hold[ci][:, HB[hf]:HB[hf + 1]],
                                lhsT=w1b16,
                                rhs=wtext[:, ci * 512 + HB[hf]:
                                          ci * 512 + HB[hf + 1]],
                                start=(hf == 0), stop=False)
                    # preload the Sqrt table while the collective runs
                    warm = pw.tile([128, 1], F32, tag="warm")
                    nc.scalar.activation(warm[:], p128[:, 42:43], AF.Sqrt)

                # ---- two pipelined AllGathers: [y-left+stats], [y-right]
                # ONE collective: the 2nd AG's pickup+floor (~10us)
                # exceeded the ~6us of work it hid.
                agA_in = pd.tile([C, 516], BF16, tag="agA_in")
                agA_out = pd.tile([C * NCORES, 516], BF16, tag="agA_out")
                nc.sync.dma_start(out=agA_in[:], in_=ag_y[:])
                nc.gpsimd.collective_compute(
                    "AllGather", ALU.bypass,
                    replica_groups=[list(range(NCORES))],
                    ins=[agA_in[:].opt()], outs=[agA_out[:].opt()])

                # ---- receive the gathers ----
                agvA = agA_out[:].rearrange("(jj p) n -> p jj n", jj=NCORES)
                statj = pc.tile([C, 32], BF16, tag="statj")
                nc.sync.dma_start(
                    out=statj[:].rearrange("p (jj s) -> p jj s", s=4),
                    in_=agvA[:, :, 384:388])
                yt_full = pb.tile([C, NT], BF16, tag="yt_full")
                ytv = yt_full[:].rearrange("p (ci n) -> p ci n", ci=NCORES)
                nc.sync.dma_start(out=ytv[:, :, 0:384],
                                  in_=agvA[:, :, 0:384])
                nc.sync.dma_start(out=ytv[:, :, 384:512],
                                  in_=agvA[:, :, 388:516])

                # ---- BN0 global stats ----
                statv = statj[:].bitcast(F32).rearrange("p (jj s) -> p s jj", s=2)
                tot = pc.tile([C, 2], F32, tag="tot")
                nc.vector.tensor_reduce(tot[:, 0:1], statv[:, 0:1, :],
                                        axis=AX.X, op=ALU.add)
                nc.vector.tensor_reduce(tot[:, 1:2], statv[:, 1:2, :],
                                        axis=AX.X, op=ALU.add)
                mom = pc.tile([C, 2], F32, tag="mom")
                nc.vector.tensor_scalar_mul(mom[:], tot[:], 1.0 / NT)
                var0 = pc.tile([C, 1], F32, tag="var0")
                nc.vector.tensor_tensor(var0[:], mom[:, 0:1], mom[:, 0:1],
                                        op=ALU.mult)
                nc.vector.scalar_tensor_tensor(
                    var0[:], mom[:, 1:2], EPS, var0[:],
                    op0=ALU.add, op1=ALU.subtract)

                def rsqrt(dst, src, p, pref):
                    # ACT Sqrt (table preloaded during the AG) + DVE recip
                    st = pc.tile([p, 1], F32, tag=pref + "s")
                    nc.scalar.activation(st[:], src[:], AF.Sqrt)
                    nc.vector.reciprocal(dst[:], st[:])

                rstd0 = pc.tile([C, 1], F32, tag="rstd0")
                rsqrt(rstd0, var0, C, "r0")
                scale0 = pc.tile([C, 1], F32, tag="scale0")
                nc.vector.tensor_tensor(scale0[:], rstd0[:], g0c,
                                        op=ALU.mult)
                w1p = pc.tile([C, CM], BF16, tag="w1p")
                nc.vector.tensor_scalar(w1p[:], w1e[0:C, :], scale0[:], None,
                                        op0=ALU.mult)
                nshift0 = pc.tile([C, 1], F32, tag="nshift0")
                nc.vector.scalar_tensor_tensor(
                    nshift0[:], mom[:, 0:1], scale0[:], be0c,
                    op0=ALU.mult, op1=ALU.subtract)

                # bias1 = w1.T @ shift0 + b1  (ones-row trick via w1e row C)
                sh1e = pc.tile([C + 1, 1], F32, tag="sh1e")
                nc.vector.memset(sh1e[C:C + 1, :], -1.0)
                nc.vector.tensor_copy(out=sh1e[0:C, :], in_=nshift0[:])
                psb1 = psT.tile([CM, 1], F32, tag="misc", name="psb1")
                nc.tensor.matmul(psb1[:], lhsT=w1e, rhs=sh1e[:],
                                 start=True, stop=True)
                bias1 = pc.tile([CM, 1], F32, tag="bias1")
                nc.vector.tensor_scalar_mul(bias1[:], psb1[:], -1.0)

                # ---- full-batch h chunks: leaky(w1'.T yT + w1.T wT + bias1)
                # processed in halves so left halves overlap AG-B
                hstat = pc.tile([CM, 32], F32, tag="hstat")
                for hf in range(2):
                    for ci in range(8):
                        if ci < 7:
                            ps = hold[ci]
                        else:
                            if hf == 0:
                                ps = psT.tile([128, 512], F32, tag="misc",
                                              name="h7")
                                hold.append(ps)
                            else:
                                ps = hold[7]
                            nc.tensor.matmul(
                                ps[:, HB[hf]:HB[hf + 1]], lhsT=w1b16,
                                rhs=wtext[:, ci * 512 + HB[hf]:
                                          ci * 512 + HB[hf + 1]],
                                start=(hf == 0), stop=False)
                        cl, cr = ci * 512 + HB[hf], ci * 512 + HB[hf + 1]
                        nc.tensor.matmul(
                            ps[:, HB[hf]:HB[hf + 1]], lhsT=w1p[:],
                            rhs=yt_full[:, cl:cr],
                            start=False, stop=(hf == 1))
                        sc = ci * 2 + hf
                        hcs = pw.tile([CM, 384], BF16, tag="hcs",
                                      name=f"hcs{sc}")
                        hcs_a = hcs[:, 0:HB[hf + 1] - HB[hf]]
                        if hf == 1 and ci % 2 == 1:
                            # spread the AG-B-gated leakys across ACT+DVE:
                            # leaky(x+b) = max(x+b, slope*(x+b)) via two-op
                            pre = pw.tile([CM, 128], F32, tag="pre",
                                          name=f"pre{sc}")
                            nc.vector.tensor_scalar(
                                pre[:], ps[:, HB[hf]:HB[hf + 1]], bias1[:],
                                None, op0=ALU.add)
                            nc.vector.scalar_tensor_tensor(
                                hcs_a, pre[:], SLOPE, pre[:],
                                op0=ALU.mult, op1=ALU.max,
                                accum_out=hstat[:, sc:sc + 1])
                        else:
                            nc.scalar.activation(
                                hcs_a, ps[:, HB[hf]:HB[hf + 1]],
                                AF.Prelu, bias=bias1[:],
                                scale=1.0, alpha=SLOPE,
                                accum_out=hstat[:, sc:sc + 1])
                        sqh = pw.tile([CM, 384], BF16, tag="sqh",
                                      name=f"sqh{sc}")
                        sqh_a = sqh[:, 0:HB[hf + 1] - HB[hf]]
                        nc.vector.scalar_tensor_tensor(
                            sqh_a, hcs_a, 1.0, hcs_a,
                            op0=ALU.mult, op1=ALU.mult,
                            accum_out=hstat[:, 16 + sc:17 + sc])

                # ---- own-slice yres + h (fully local, no rank indexing) ----
                yres = pb.tile([C, 512], F32, tag="yres")
                with tc.high_priority():
                    nc.vector.tensor_scalar(yres[:, 0:384], ag_y[:, 0:384],
                                            scale0[:], nshift0[:],
                                            op0=ALU.mult, op1=ALU.subtract)
                    nc.vector.tensor_scalar(yres[:, 384:512],
                                            ag_y[:, 388:516],
                                            scale0[:], nshift0[:],
                                            op0=ALU.mult, op1=ALU.subtract)
                    nc.vector.tensor_tensor(yres[:], yres[:], wto,
                                            op=ALU.add)
                pso = psR.tile([128, 512], F32, tag="hr")
                nc.tensor.matmul(pso[:], lhsT=w1b16, rhs=wtob,
                                 start=True, stop=False)
                nc.tensor.matmul(pso[:, 0:384], lhsT=w1p[:],
                                 rhs=ag_y[:, 0:384],
                                 start=False, stop=False)
                nc.tensor.matmul(pso[:, 384:512], lhsT=w1p[:],
                                 rhs=ag_y[:, 388:516],
                                 start=False, stop=True)
                h_own = pb.tile([CM, 512], BF16, tag="h_own")
                nc.scalar.activation(h_own[:], pso[:], AF.Prelu,
                                     bias=bias1[:], scale=1.0, alpha=SLOPE)

                # ---- BN1 stats + fold into w2 ----
                tot1 = pc.tile([CM, 2], F32, tag="tot1")
                nc.vector.tensor_reduce(tot1[:, 0:1], hstat[:, 0:16],
                                        axis=AX.X, op=ALU.add)
                nc.vector.tensor_reduce(tot1[:, 1:2], hstat[:, 16:32],
                                        axis=AX.X, op=ALU.add)
                mom1 = pc.tile([CM, 2], F32, tag="mom1")
                nc.vector.tensor_scalar_mul(mom1[:], tot1[:], 1.0 / NT)
                var1 = pc.tile([CM, 1], F32, tag="var1")
                nc.vector.tensor_tensor(var1[:], mom1[:, 0:1], mom1[:, 0:1],
                                        op=ALU.mult)
                nc.vector.scalar_tensor_tensor(
                    var1[:], mom1[:, 1:2], EPS, var1[:],
                    op0=ALU.add, op1=ALU.subtract)
                rstd1 = pc.tile([CM, 1], F32, tag="rstd1")
                rsqrt(rstd1, var1, CM, "r1")
                scale1 = pc.tile([CM, 1], F32, tag="scale1")
                nc.vector.tensor_tensor(scale1[:], rstd1[:], g1c,
                                        op=ALU.mult)
                nshift1 = pc.tile([CM, 1], F32, tag="nshift1")
                nc.vector.scalar_tensor_tensor(
                    nshift1[:], mom1[:, 0:1], scale1[:], be1c,
                    op0=ALU.mult, op1=ALU.subtract)
                w2p = pc.tile([CM, C], BF16, tag="w2p")
                nc.vector.tensor_scalar(w2p[:], w2, scale1[:], None,
                                        op0=ALU.mult)
                psb2 = psT.tile([C, 1], F32, tag="misc", name="psb2")
                nc.tensor.matmul(psb2[:], lhsT=w2, rhs=nshift1[:],
                                 start=True, stop=True)
                bias2 = pc.tile([C, 1], F32, tag="bias2")
                nc.vector.tensor_tensor(bias2[:], b2c, psb2[:],
                                        op=ALU.subtract)

                # ---- delta = w2'.T @ h_own + bias2; out = yres + delta ----
                psd = psT.tile([C, 512], F32, tag="misc", name="psd")
                nc.tensor.matmul(psd[:], lhsT=w2p[:], rhs=h_own[:],
                                 start=True, stop=True)
                out_sb = pw.tile([C, 512], F32, tag="outsb")
                nc.vector.scalar_tensor_tensor(
                    out_sb[:], psd[:], bias2[:], yres[:],
                    op0=ALU.add, op1=ALU.add)
                nc.scalar.dma_start(out=out_d, in_=out_sb[:])


    nc.compile()
    return nc


_NC_CACHE = {}


def _get_module():
    if "nc" not in _NC_CACHE:
        _NC_CACHE["nc"] = _build_module()
    return _NC_CACHE["nc"]


BF = ml_dtypes.bfloat16


def _split_hi_lo(x):
    hi = x.astype(BF)
    lo = (x - hi.astype(np.float32)).astype(BF)
    return hi, lo


def _host_prep(inputs):
    pos = np.asarray(inputs["positions"], np.float32)
    w = np.asarray(inputs["weights"], np.float32)
    kp = np.asarray(inputs["kernel_pos"], np.float32)
    cw = np.asarray(inputs["conv_w"], np.float32)
    posb = pos.reshape(NB, N, 2)
    wb = w.reshape(NB, N, C)
    kk2 = 0.5 * (kp ** 2).sum(1)                       # [9]
    cwall = cw.transpose(1, 0, 2).reshape(C, KC).astype(BF)
    wTfull = np.ascontiguousarray(w.T)                 # [32, 4096]
    w1 = np.asarray(inputs["w1"], np.float32)
    w2 = np.asarray(inputs["w2"], np.float32)
    w1e = np.concatenate(
        [w1, np.asarray(inputs["b1"], np.float32).reshape(1, CM)], axis=0)
    p128 = np.zeros((128, 44), np.float32)
    p128[:, 8:40] = w2
    p128[:, 40] = np.asarray(inputs["bn1_gamma"], np.float32)
    p128[:, 41] = np.asarray(inputs["bn1_beta"], np.float32)
    p128[:, 42] = np.int32(1).view(np.float32)
    p128[:, 43] = np.int32(MAGIC).view(np.float32)

    in_maps = []
    for j in range(NCORES):
        b, off = j // 2, (j % 2) * 512
        p = posb[b]
        pi = p[off:off + 512]
        xh, xl = _split_hi_lo(p[:, 0])
        yh, yl = _split_hi_lo(p[:, 1])
        one = np.ones(N, BF)
        pn8 = np.stack([xh, xh, xl, yh, yh, yl, one, one])
        bias = -0.5 * (pi ** 2).sum(1)
        bh, bl = _split_hi_lo(bias)
        xih, xil = _split_hi_lo(pi[:, 0])
        yih, yil = _split_hi_lo(pi[:, 1])
        pi8 = np.stack([xih, xil, xih, yih, yil, yih, bh, bl])
        packp = np.concatenate([pn8, pi8], axis=1)          # [8, 1536]

        packw = np.empty((C, 6048), BF)
        packw[:, 0:1024] = wb[b].T.astype(BF)
        packw[:, 1024:5120] = wTfull.astype(BF)
        packw[:, 5120:5632] = wTfull[:, j * 512:(j + 1) * 512].astype(BF)
        packw[:, 5632:5760] = w1.astype(BF)
        packw[:, 5760:6048] = cwall

        p128j = p128.copy()
        p128j[:, 0:8] = (-0.5 * (p ** 2).sum(1)).reshape(8, 128).T
        dotn = (p @ kp.T).astype(np.float32)            # [1024, 9]
        Bmat = np.exp(-dotn - kk2[None, :]).astype(np.float32)
        Amat = np.exp((pi @ kp.T).astype(np.float32)).astype(np.float32)
        packf = np.empty((128, 3456), BF)
        packf[:, 0:8 * KC] = np.broadcast_to(
            Bmat.reshape(8, 128, K, 1).transpose(1, 0, 2, 3),
            (128, 8, K, C)).reshape(128, 8 * KC).astype(BF)
        packf[:, 8 * KC:] = np.broadcast_to(
            Amat.reshape(4, 128, K, 1).transpose(1, 0, 2, 3),
            (128, 4, K, C)).reshape(128, 4 * KC).astype(BF)

        packs = np.zeros((C + 1, 644), np.float32)
        packs[:, 0:CM] = w1e
        packs[0:C, 128] = np.asarray(inputs["bn_gamma"], np.float32)
        packs[0:C, 129] = np.asarray(inputs["bn_beta"], np.float32)
        packs[0:C, 130] = np.asarray(inputs["b2"], np.float32)
        packs[0:C, 131:643] = wTfull[:, j * 512:(j + 1) * 512]

        in_maps.append(dict(
            packp=np.ascontiguousarray(packp),
            packw=np.ascontiguousarray(packw),
            packf=np.ascontiguousarray(packf),
            packs=np.ascontiguousarray(packs),
            p128=np.ascontiguousarray(p128j)))
    return in_maps


def _run(inputs, trace=False):
    nc = _get_module()
    in_maps = _host_prep(inputs)
    res = run_bass_kernel_spmd(nc, in_maps, core_ids=list(range(NCORES)),
                               trace=trace)
    out = np.concatenate([np.asarray(res.results[j]["out"])
                          for j in range(NCORES)], axis=1)   # [32, 4096]
    return np.ascontiguousarray(out.T), res


def kernel(**inputs):
    out, _ = _run(inputs, trace=False)
    return out
